# revision 26
# baseline (speedup 1.0000x reference)
"""Trainium2 Bass kernel for the CCG supertagger BERT model.

Data-parallel over batch: 16 samples -> 8 cores x 2 samples.
Activations kept transposed [H (6 chunks of 128), T=512 tokens] in SBUF.
fp32r matmuls for the residual-stream projections; bf16 for attention
internals / Wo2 / head-w2 (fp32->bf16 cast done inside gpsimd DMA).
"""
import numpy as np

import concourse.bass as bass
import concourse.tile as tile
from concourse import bacc, mybir
from concourse.bass_utils import run_bass_kernel_spmd
from concourse.masks import make_identity

F32 = mybir.dt.float32
F32R = mybir.dt.float32r
BF16 = mybir.dt.bfloat16
I32 = mybir.dt.int32
AF = mybir.ActivationFunctionType
ALU = mybir.AluOpType

B, S, W = 16, 256, 128
V, H, L, NH, DH, FF, C = 30522, 768, 12, 12, 64, 3072, 425
EPS = 1e-12
N_CORES = 8
BPC = B // N_CORES          # samples per core
T = BPC * S                 # tokens per core (512)
HC = H // 128               # 6
FFC = FF // 128             # 24
TC = T // 128               # 4 token chunks
M1 = 1024
M1C = M1 // 128             # 8
CPAD = 448                  # padded class dim for sbuf tiles


DEBUG_TAPS = False


def build_program(n_layers=L):
    nc = bacc.Bacc("TRN2", target_bir_lowering=False, debug=False,
                   num_devices=N_CORES)

    dt_ = lambda name, shape, dt, kind: nc.dram_tensor(name, shape, dt, kind=kind).ap()
    # per-core sharded inputs
    enc = dt_("enc", [T, 1], I32, "ExternalInput")
    ab = dt_("ab", [T], F32, "ExternalInput")            # attn bias (per key pos)
    pmat = dt_("pmat", [T, 128], F32, "ExternalInput")   # pooling matrices
    # replicated model inputs
    word_emb = dt_("word_emb", [V, H], F32, "ExternalInput")
    pos_emb = dt_("pos_emb", [S, H], F32, "ExternalInput")
    type_emb = dt_("type_emb", [1, H], F32, "ExternalInput")
    emb_ln_s = dt_("emb_ln_s", [H], F32, "ExternalInput")
    emb_ln_b = dt_("emb_ln_b", [H], F32, "ExternalInput")
    Wq = dt_("Wq", [L, H, H], F32, "ExternalInput")
    bq = dt_("bq", [L, H], F32, "ExternalInput")
    Wk = dt_("Wk", [L, H, H], F32, "ExternalInput")
    bk = dt_("bk", [L, H], F32, "ExternalInput")
    Wv = dt_("Wv", [L, H, H], F32, "ExternalInput")
    bv = dt_("bv", [L, H], F32, "ExternalInput")
    Wo = dt_("Wo", [L, H, H], F32, "ExternalInput")
    bo = dt_("bo", [L, H], F32, "ExternalInput")
    ln1_s = dt_("ln1_s", [L, H], F32, "ExternalInput")
    ln1_b = dt_("ln1_b", [L, H], F32, "ExternalInput")
    Wi = dt_("Wi", [L, H, FF], F32, "ExternalInput")
    bi = dt_("bi", [L, FF], F32, "ExternalInput")
    Wo2 = dt_("Wo2", [L, FF, H], F32, "ExternalInput")
    bo2 = dt_("bo2", [L, H], F32, "ExternalInput")
    ln2_s = dt_("ln2_s", [L, H], F32, "ExternalInput")
    ln2_b = dt_("ln2_b", [L, H], F32, "ExternalInput")
    w1 = dt_("w1", [H, M1], F32, "ExternalInput")
    b1 = dt_("b1", [M1], F32, "ExternalInput")
    w2 = dt_("w2", [M1, C], F32, "ExternalInput")
    b2 = dt_("b2", [C], F32, "ExternalInput")
    cones = dt_("cones", [128, 1], F32, "ExternalInput")   # column of ones
    crow = dt_("crow", [1, 128], F32, "ExternalInput")     # row of ones
    ceps = dt_("ceps", [128, 1], F32, "ExternalInput")     # EPS constant
    out_d = dt_("out", [T, C], F32, "ExternalOutput")
    if DEBUG_TAPS:
        dbg_x0 = dt_("dbg_x0", [H, T], F32, "ExternalOutput")
        dbg_q = dt_("dbg_q", [H, T], BF16, "ExternalOutput")
        dbg_ctx = dt_("dbg_ctx", [T, H], BF16, "ExternalOutput")
        dbg_a = dt_("dbg_a", [H, T], F32, "ExternalOutput")
        dbg_h2 = dt_("dbg_h2", [H, T], F32, "ExternalOutput")
        dbg_f2 = dt_("dbg_f2", [H, T], F32, "ExternalOutput")
        dbg_x1 = dt_("dbg_x1", [H, T], F32, "ExternalOutput")
        dbg_exp0 = dt_("dbg_exp0", [128, 256], BF16, "ExternalOutput")
        dbg_exp1 = dt_("dbg_exp1", [128, 256], BF16, "ExternalOutput")
        dbg_rec0 = dt_("dbg_rec0", [128, 1], F32, "ExternalOutput")
        dbg_v = dt_("dbg_v", [T, H], BF16, "ExternalOutput")

    with tile.TileContext(nc) as tc:
        _emit(nc, tc, n_layers, locals())
    nc.compile()
    return nc


def _emit(nc, tc, n_layers, d):
    from contextlib import ExitStack
    ctx = ExitStack()
    with ctx:
        _emit_body(nc, tc, n_layers, d, ctx)


def _emit_body(nc, tc, n_layers, d, ctx):
    pool = lambda name, bufs, space="SBUF": ctx.enter_context(
        tc.tile_pool(name=name, bufs=bufs, space=space))

    p_xt = pool("xt", 3)          # [128, HC, 512] f32r residual-stream acts
    p_qk = pool("qk", 3)          # [128, HC, 512] bf16 (QT, KT, VT, reluT)
    p_v = pool("v", 1)            # [128, TC, 768] bf16 token-major V / f2_sb
    p_ctx = pool("ctxp", 1)       # [128, HC, 512] f32r ctxT
    p_exp = pool("exp", 5)        # [128, 256] bf16 exp tiles
    p_scr = pool("scr", 3)        # [128, 512] f32 scratch
    p_gel = pool("gel", 1)        # [128, HC, 512] bf16 gelu quarter / emb x0 / w1
    p_f2a = pool("f2a", 1)        # [128, HC, 512] f32 FFN accumulator
    p_wim = pool("wim", 3)        # [128, HC, 128] f32r m-major weight bands
    p_wb = pool("wb", 4)          # [128, 768] bf16 Wo2 k-bands
    p_f1 = pool("f1", 1)          # [128, M1C, 512] bf16 head f1relu
    p_w2 = pool("w2p", 1)         # [128, M1C, 425] bf16 head w2
    p_f2 = pool("f2sb", 1)        # [128, TC, CPAD] f32r head logits token-major
    p_bias = pool("bias", 6)      # [128, 24] f32 per-partition bias/scale tiles
    p_vec = pool("vec", 3)        # [1, 512] f32 LN stat vectors
    p_vec2 = pool("vec2", 1)      # [1, 1024] f32 (rstd | -mu*rstd)
    p_lnbc = pool("lnbc", 2)      # [128, 1024] f32 broadcast LN stats
    p_dram = pool("dram", 2, "DRAM")
    p_row = pool("row", 2)        # [1, 768] f32r bias rows
    p_sm = pool("sm", 2)          # small per-chunk scalars
    p_cst = pool("cst", 1)        # constants
    p_emb = pool("emb", 1)        # [128, TC, 768] f32 embedding workspace
    p_pos = pool("pos", 1)

    ps_mm = pool("ps_mm", 3, "PSUM")    # [128, 512]
    ps_sc = pool("ps_sc", 2, "PSUM")    # [128, 256] scores / [1, 512] LN stats
    ps_cx = pool("ps_cx", 2, "PSUM")    # [128, 64] ctx
    ps_su = pool("ps_su", 1, "PSUM")    # [128, 1] softmax sums

    enc, ab, pmat = d["enc"], d["ab"], d["pmat"]
    word_emb, pos_emb, type_emb = d["word_emb"], d["pos_emb"], d["type_emb"]
    emb_ln_s, emb_ln_b = d["emb_ln_s"], d["emb_ln_b"]
    out_d = d["out_d"]

    # ---- constants ----
    ident = p_cst.tile([128, 128], BF16, tag="ident")
    make_identity(nc, ident[:])
    ones_c = p_cst.tile([128, 1], F32R, tag="ones_c")
    nc.sync.dma_start(ones_c[:], d["cones"][:].bitcast(F32R))
    ones_cb = p_cst.tile([128, 1], BF16, tag="ones_cb")
    nc.gpsimd.dma_start(ones_cb[:], d["cones"][:])
    ones_r = p_cst.tile([1, 128], F32R, tag="ones_r")
    nc.sync.dma_start(ones_r[:], d["crow"][:].bitcast(F32R))
    eps_t = p_cst.tile([128, 1], F32, tag="eps")
    nc.sync.dma_start(eps_t[:], d["ceps"][:])

    # attn bias as [128, TC]
    ab_t = p_cst.tile([128, TC], F32, tag="ab")
    nc.sync.dma_start(ab_t[:], ab.rearrange("(c p) -> p c", p=128))

    def ln_pair(ap_s, ap_b, tag):
        t = p_bias.tile([128, 2 * HC], F32, tag="bias")
        nc.sync.dma_start(t[:, 0:HC], ap_s.rearrange("(c p) -> p c", p=128))
        nc.sync.dma_start(t[:, HC:2 * HC], ap_b.rearrange("(c p) -> p c", p=128))
        return t

    # =============== embedding ===============
    x0 = p_emb.tile([128, TC, H], F32, tag="x0")
    for c in range(TC):
        idx_t = p_sm.tile([128, 1], I32, tag="idx")
        nc.sync.dma_start(idx_t[:], enc[128 * c:128 * (c + 1), :])
        nc.gpsimd.indirect_dma_start(
            out=x0[:, c, :], out_offset=None, in_=word_emb[:],
            in_offset=bass.IndirectOffsetOnAxis(ap=idx_t[:, :1], axis=0))
    pos_t = p_pos.tile([128, 2, H], F32, tag="pos")
    nc.sync.dma_start(pos_t[:, 0, :], pos_emb[0:128, :])
    nc.sync.dma_start(pos_t[:, 1, :], pos_emb[128:256, :])
    typ_t = p_pos.tile([128, H], F32, tag="typ")
    nc.sync.dma_start(typ_t[:], type_emb[0:1, :].partition_broadcast(128)[:, 0, :])

    emb_sb = ln_pair(emb_ln_s, emb_ln_b, "embln")
    X = p_xt.tile([128, HC, T], F32R, tag="xt")
    for c in range(TC):
        xc = x0[:, c, :]
        nc.vector.tensor_tensor(out=xc, in0=xc, in1=pos_t[:, c % 2, :], op=ALU.add)
        nc.vector.tensor_tensor(out=xc, in0=xc, in1=typ_t[:], op=ALU.add)
        # layernorm over free dim (token-major)
        su = p_sm.tile([128, 4], F32, tag="stat")
        nc.vector.reduce_sum(out=su[:, 0:1], in_=xc, axis=mybir.AxisListType.X)
        sq = p_scr.tile([128, H], F32, tag="scr")
        nc.scalar.activation(sq[:], xc, AF.Square, accum_out=su[:, 1:2])
        st = p_sm.tile([128, 4], F32, tag="stat2")
        nc.vector.tensor_scalar_mul(st[:, 0:1], su[:, 0:1], 1.0 / H)      # mu
        nc.vector.tensor_scalar_mul(st[:, 1:2], su[:, 1:2], 1.0 / H)      # m2
        nc.vector.tensor_tensor(out=st[:, 2:3], in0=st[:, 0:1], in1=st[:, 0:1], op=ALU.mult)
        nc.vector.tensor_tensor(out=st[:, 3:4], in0=st[:, 1:2], in1=st[:, 2:3], op=ALU.subtract)
        sd = p_sm.tile([128, 2], F32, tag="stat3")
        nc.scalar.activation(sd[:, 0:1], st[:, 3:4], AF.Sqrt, bias=eps_t[:, 0:1])
        nc.vector.reciprocal(sd[:, 1:2], sd[:, 0:1])
        # x_hat = (x - mu) * rstd
        nc.vector.tensor_scalar(out=xc, in0=xc, scalar1=st[:, 0:1], scalar2=sd[:, 1:2],
                                op0=ALU.subtract, op1=ALU.mult)
        # transpose into X_T, fused scale/bias
        for k in range(HC):
            pt = ps_mm.tile([128, 128], BF16, tag="ps_mm")
            xcb = p_scr.tile([128, 128], BF16, tag="scrb")
            nc.vector.tensor_copy(xcb[:], xc[:, 128 * k:128 * (k + 1)])
            nc.tensor.transpose(pt[:], xcb[:], ident[:])
            nc.scalar.activation(X[:, k, 128 * c:128 * (c + 1)], pt[:],
                                 AF.Identity, scale=emb_sb[:, k:k + 1],
                                 bias=emb_sb[:, HC + k:HC + k + 1])

    def tap(name, tile_, nchunks, dt=F32):
        if not DEBUG_TAPS or name not in d:
            return
        ap = d[name]
        for k in range(nchunks):
            nc.sync.dma_start(ap[128 * k:128 * (k + 1), :],
                              tile_[:, k, :] if dt is None else tile_[:, k, :].bitcast(dt))

    tap("dbg_x0", X, HC)
    # =============== transformer layers ===============
    for l in range(n_layers):
        X = _layer(nc, tc, d, l, X, dict(
            p_xt=p_xt, p_qk=p_qk, p_v=p_v, p_ctx=p_ctx, p_exp=p_exp,
            p_scr=p_scr, p_gel=p_gel, p_f2a=p_f2a, p_wim=p_wim, p_wb=p_wb,
            p_bias=p_bias, p_vec=p_vec, p_vec2=p_vec2, p_lnbc=p_lnbc, p_dram=p_dram, p_row=p_row, p_sm=p_sm,
            ps_mm=ps_mm, ps_sc=ps_sc, ps_cx=ps_cx, ps_su=ps_su,
            ones_c=ones_c, ones_cb=ones_cb, ones_r=ones_r, ab_t=ab_t, eps_t=eps_t, tap=tap, dd=d,
            ident=ident, ln_pair=ln_pair))

    if DEBUG_TAPS:
        tap("dbg_x1", X, HC)
    # =============== head ===============
    _head(nc, tc, d, X, dict(
        p_qk=p_qk, p_v=p_v, p_f1=p_f1, p_w2=p_w2, p_f2=p_f2, p_gel=p_gel,
        p_scr=p_scr, p_bias=p_bias, p_wim=p_wim, p_sm=p_sm, p_row=p_row,
        ps_mm=ps_mm, ones_r=ones_r, pmat=pmat, out_d=out_d))


def _wband(nc, p_wim, w_ap, l, m, n_k=HC, dt=F32R):
    """m-major weight band: [128, n_k, 128] with (p, k, f) <- W[l, 128k+p, 128m+f]."""
    t = p_wim.tile([128, n_k, 128], dt, tag="wim")
    src = w_ap[l].rearrange("(k p) f -> p k f", p=128)[:, :, 128 * m:128 * (m + 1)]
    if dt == F32R:
        nc.sync.dma_start(t[:], src.bitcast(F32R))
    else:
        nc.gpsimd.dma_start(t[:], src)
    return t


def _layer(nc, tc, d, l, X, e):
    p_xt, p_qk, p_v, p_ctx, p_exp = e["p_xt"], e["p_qk"], e["p_v"], e["p_ctx"], e["p_exp"]
    p_scr, p_gel, p_f2a, p_wim, p_wb = e["p_scr"], e["p_gel"], e["p_f2a"], e["p_wim"], e["p_wb"]
    p_bias, p_vec, p_row, p_sm = e["p_bias"], e["p_vec"], e["p_row"], e["p_sm"]
    ps_mm, ps_sc, ps_cx, ps_su = e["ps_mm"], e["ps_sc"], e["ps_cx"], e["ps_su"]
    ones_c, ones_cb, ones_r, ab_t = e["ones_c"], e["ones_cb"], e["ones_r"], e["ab_t"]
    dd = e["dd"]
    ident, ln_pair = e["ident"], e["ln_pair"]

    qkvo_b = p_bias.tile([128, 4 * HC], F32, tag="bias")
    for i, bap in enumerate([d["bq"], d["bk"], d["bv"], d["bo"]]):
        nc.sync.dma_start(qkvo_b[:, i * HC:(i + 1) * HC],
                          bap[l].rearrange("(c p) -> p c", p=128))

    # ---- Q, K, V projections (transposed layout), V then transposed to token-major
    QT = p_qk.tile([128, HC, T], BF16, tag="qk")
    KT = p_qk.tile([128, HC, T], BF16, tag="qk")
    VT = p_qk.tile([128, HC, T], BF16, tag="qk")
    for wi_, (w_ap, dst, boff) in enumerate([(d["Wq"], QT, 0), (d["Wk"], KT, HC),
                                             (d["Wv"], VT, 2 * HC)]):
        for m in range(HC):
            band = _wband(nc, p_wim, w_ap, l, m)
            pm_ = ps_mm.tile([128, T], F32, tag="ps_mm")
            for k in range(HC):
                nc.tensor.matmul(pm_[:], band[:, k, :], X[:, k, :],
                                 start=(k == 0), stop=(k == HC - 1))
            nc.scalar.activation(dst[:, m, :], pm_[:], AF.Identity,
                                 bias=qkvo_b[:, boff + m:boff + m + 1])

    if l == 0:
        e["tap"]("dbg_q", QT, HC, BF16)
    # V -> token-major [128, TC, H] bf16 via PE transpose
    Vtok = p_v.tile([128, TC, H], BF16, tag="v")
    for c in range(TC):
        for k in range(HC):
            pt = ps_mm.tile([128, 128], BF16, tag="ps_mm")
            nc.tensor.transpose(pt[:], VT[:, k, 128 * c:128 * (c + 1)], ident[:])
            nc.scalar.activation(Vtok[:, c, 128 * k:128 * (k + 1)], pt[:], AF.Copy)

    if l == 0:
        e["tap"]("dbg_v", Vtok, TC, BF16)
    # ---- attention ----
    # ctx accumulated token-major with a fused softmax-sum column, then
    # transposed back to [H, T] layout for the O-projection.
    ctok = p_ctx.tile([128, TC, H], BF16, tag="ctxp")
    for s in range(BPC):
        for h in range(NH):
            kc, po = h // 2, 64 * (h % 2)
            exp_t = [None, None]
            for j in range(2):
                psc = ps_sc.tile([128, 256], F32, tag="ps_sc")
                nc.tensor.matmul(
                    psc[:],
                    KT[po:po + 64, kc, 256 * s + 128 * j:256 * s + 128 * (j + 1)],
                    QT[po:po + 64, kc, 256 * s:256 * (s + 1)],
                    start=True, stop=True)
                et = p_exp.tile([128, 256], BF16, tag="exp")
                nc.scalar.activation(et[:], psc[:], AF.Exp, scale=0.125,
                                     bias=ab_t[:, 2 * s + j:2 * s + j + 1])
                exp_t[j] = et
            if l == 0 and s == 0 and h == 0 and "dbg_exp0" in dd:
                nc.sync.dma_start(dd["dbg_exp0"][:], exp_t[0][:])
                nc.sync.dma_start(dd["dbg_exp1"][:], exp_t[1][:])
            for i in range(2):            # query chunks
                pcx = ps_cx.tile([128, 64], F32, tag="ps_cx")
                psu = ps_su.tile([128, 1], F32, tag="ps_su")
                for j in range(2):
                    lhs = exp_t[j][:, 128 * i:128 * (i + 1)]
                    nc.tensor.matmul(pcx[:], lhs,
                                     Vtok[:, 2 * s + j, 64 * h:64 * h + 64],
                                     start=(j == 0), stop=(j == 1))
                    nc.tensor.matmul(psu[:], lhs, ones_cb[:],
                                     start=(j == 0), stop=(j == 1))
                rec = p_sm.tile([128, 1], F32, tag="rec")
                nc.vector.reciprocal(rec[:], psu[:])
                if l == 0 and s == 0 and h == 0 and i == 0 and "dbg_rec0" in dd:
                    nc.sync.dma_start(dd["dbg_rec0"][:], rec[:])
                nc.vector.tensor_scalar_mul(
                    ctok[:, 2 * s + i, 64 * h:64 * h + 64], pcx[:], rec[:])
    if l == 0:
        e["tap"]("dbg_ctx", ctok, TC, BF16)
    # transpose ctx back to [H, T]
    ctxT = p_qk.tile([128, HC, T], BF16, tag="qk")
    for c in range(TC):
        for k in range(HC):
            pt = ps_mm.tile([128, 128], BF16, tag="ps_mm")
            nc.tensor.transpose(pt[:], ctok[:, c, 128 * k:128 * (k + 1)], ident[:])
            nc.scalar.activation(ctxT[:, k, 128 * c:128 * (c + 1)], pt[:], AF.Copy)

    # ---- O-projection + residual + LN1 ----
    ln1 = ln_pair(d["ln1_s"][l], d["ln1_b"][l], "ln1")
    A = p_xt.tile([128, HC, T], F32R, tag="xt")
    for m in range(HC):
        band = _wband(nc, p_wim, d["Wo"], l, m, dt=BF16)
        pm_ = ps_mm.tile([128, T], F32, tag="ps_mm")
        for k in range(HC):
            nc.tensor.matmul(pm_[:], band[:, k, :], ctxT[:, k, :],
                             start=(k == 0), stop=(k == HC - 1))
        t1 = p_scr.tile([128, T], F32, tag="scr")
        nc.scalar.activation(t1[:], pm_[:], AF.Identity,
                             bias=qkvo_b[:, 3 * HC + m:3 * HC + m + 1])
        nc.vector.tensor_tensor(out=A[:, m, :], in0=t1[:], in1=X[:, m, :].bitcast(F32),
                                op=ALU.add)
    if l == 0:
        e["tap"]("dbg_a", A, HC)
    H2 = _ln_t(nc, A, ln1, e)
    if l == 0:
        e["tap"]("dbg_h2", H2, HC)

    # ---- FFN (quarter passes over FF) ----
    ln2 = ln_pair(d["ln2_s"][l], d["ln2_b"][l], "ln2")
    bi_t = p_bias.tile([128, FFC], F32, tag="bias")
    nc.sync.dma_start(bi_t[:], d["bi"][l].rearrange("(c p) -> p c", p=128))
    bo2_t = p_bias.tile([128, HC], F32, tag="bias")
    nc.sync.dma_start(bo2_t[:], d["bo2"][l].rearrange("(c p) -> p c", p=128))

    F2 = p_f2a.tile([128, HC, T], F32, tag="f2a")
    NQ = 4
    QK = FFC // NQ                      # 6 ff-chunks per quarter
    for q in range(NQ):
        gel = p_gel.tile([128, QK, T], BF16, tag="gel")
        for mq in range(QK):
            m = q * QK + mq
            band = _wband(nc, p_wim, d["Wi"], l, m)
            pm_ = ps_mm.tile([128, T], F32, tag="ps_mm")
            for k in range(HC):
                nc.tensor.matmul(pm_[:], band[:, k, :], H2[:, k, :],
                                 start=(k == 0), stop=(k == HC - 1))
            nc.scalar.activation(gel[:, mq, :], pm_[:], AF.Gelu,
                                 bias=bi_t[:, m:m + 1])
        for o in range(HC):
            pm_ = ps_mm.tile([128, T], F32, tag="ps_mm")
            for kq in range(QK):
                m = q * QK + kq
                wb = p_wb.tile([128, 768], BF16, tag="wb")
                nc.gpsimd.dma_start(wb[:], d["Wo2"][l, 128 * m:128 * (m + 1), :])
                nc.tensor.matmul(pm_[:], wb[:, 128 * o:128 * (o + 1)], gel[:, kq, :],
                                 start=(kq == 0), stop=(kq == QK - 1))
            if q == 0:
                nc.scalar.activation(F2[:, o, :], pm_[:], AF.Copy)
            else:
                nc.vector.tensor_tensor(out=F2[:, o, :], in0=F2[:, o, :], in1=pm_[:],
                                        op=ALU.add)
    # residual + bias
    Apre = p_xt.tile([128, HC, T], F32R, tag="xt")
    for o in range(HC):
        t1 = p_scr.tile([128, T], F32, tag="scr")
        nc.vector.tensor_scalar_add(t1[:], F2[:, o, :], bo2_t[:, o:o + 1])
        nc.vector.tensor_tensor(out=Apre[:, o, :], in0=t1[:],
                                in1=H2[:, o, :].bitcast(F32), op=ALU.add)
    if l == 0:
        e["tap"]("dbg_f2", Apre, HC)
    return _ln_t(nc, Apre, ln2, e)


def _ln_t(nc, A, ln_sb, e):
    """LayerNorm over the partition (H) dim for transposed activations.
    A: [128, HC, T] f32r tile. ln_sb: [128, 2*HC] (scale | bias).
    Returns new [128, HC, T] f32r tile."""
    p_xt, p_scr = e["p_xt"], e["p_scr"]
    ps_mm = e["ps_mm"]
    ones_c, ones_r = e["ones_c"], e["ones_r"]

    pmean = e["ps_sc"].tile([1, T], F32, tag="ps_sc")
    for k in range(HC):
        nc.tensor.matmul(pmean[:], ones_c[:], A[:, k, :],
                         start=(k == 0), stop=(k == HC - 1))
    psq = e["ps_sc"].tile([1, T], F32, tag="ps_sc")
    for k in range(HC):
        sq = p_scr.tile([128, T], F32R, tag="scr")
        nc.scalar.activation(sq[:], A[:, k, :].bitcast(F32), AF.Square)
        nc.tensor.matmul(psq[:], ones_c[:], sq[:],
                         start=(k == 0), stop=(k == HC - 1))
    va = e["p_vec"].tile([1, T], F32, tag="vec")   # mu
    vb = e["p_vec"].tile([1, T], F32, tag="vec")   # m2 -> var
    vc = e["p_vec"].tile([1, T], F32, tag="vec")   # musq -> sd -> mu*rstd
    nc.vector.tensor_scalar_mul(va[:], pmean[:], 1.0 / H)
    nc.vector.tensor_scalar_mul(vb[:], psq[:], 1.0 / H)
    nc.vector.tensor_tensor(out=vc[:], in0=va[:], in1=va[:], op=ALU.mult)
    nc.vector.tensor_tensor(out=vb[:], in0=vb[:], in1=vc[:], op=ALU.subtract)
    nc.scalar.activation(vc[:], vb[:], AF.Sqrt, bias=e["eps_t"][0:1, 0:1])
    vec2 = e["p_vec2"].tile([1, 2 * T], F32, tag="vec2")
    rstd, nmr = vec2[:, 0:T], vec2[:, T:2 * T]
    nc.vector.reciprocal(rstd, vc[:])
    nc.vector.tensor_tensor(out=vc[:], in0=va[:], in1=rstd, op=ALU.mult)
    nc.vector.tensor_scalar_mul(nmr, vc[:], -1.0)
    # broadcast rstd and -mu*rstd across partitions via a DRAM bounce
    dscr = e["p_dram"].tile([1, 2 * T], F32, tag="lnscr")
    nc.sync.dma_start(dscr[:], vec2[:])
    bc = e["p_lnbc"].tile([128, 2 * T], F32, tag="lnbc")
    nc.sync.dma_start(bc[:], dscr[:].partition_broadcast(128)[:, 0, :])
    out = p_xt.tile([128, HC, T], F32R, tag="xt")
    for k in range(HC):
        t2 = p_scr.tile([128, T], F32, tag="scr")
        nc.vector.tensor_tensor(out=t2[:], in0=A[:, k, :].bitcast(F32),
                                in1=bc[:, 0:T], op=ALU.mult)
        nc.vector.tensor_tensor(out=t2[:], in0=t2[:], in1=bc[:, T:2 * T], op=ALU.add)
        nc.scalar.activation(out[:, k, :], t2[:], AF.Identity,
                             scale=ln_sb[:, k:k + 1], bias=ln_sb[:, HC + k:HC + k + 1])
    return out


def _head(nc, tc, d, X, e):
    p_qk, p_v, p_f1, p_w2, p_f2 = e["p_qk"], e["p_v"], e["p_f1"], e["p_w2"], e["p_f2"]
    p_gel, p_scr, p_bias, p_wim, p_sm = e["p_gel"], e["p_scr"], e["p_bias"], e["p_wim"], e["p_sm"]
    ps_mm, ones_r = e["ps_mm"], e["ones_r"]
    pmat, out_d = e["pmat"], e["out_d"]

    # relu(x) transposed, bf16
    reluT = p_qk.tile([128, HC, T], BF16, tag="qk")
    for k in range(HC):
        nc.scalar.activation(reluT[:, k, :], X[:, k, :].bitcast(F32), AF.Relu)
    # f1 = relu(relu(x) @ w1 + b1), transposed layout [M1C, T]
    b1_t = p_bias.tile([128, M1C], F32, tag="bias")
    nc.sync.dma_start(b1_t[:], d["b1"].rearrange("(c p) -> p c", p=128))
    w1_t = p_gel.tile([128, HC, M1], BF16, tag="gel")
    nc.gpsimd.dma_start(w1_t[:], d["w1"].rearrange("(k p) f -> p k f", p=128))
    f1 = p_f1.tile([128, M1C, T], BF16, tag="f1")
    for m in range(M1C):
        pm_ = ps_mm.tile([128, T], F32, tag="ps_mm")
        for k in range(HC):
            nc.tensor.matmul(pm_[:], w1_t[:, k, 128 * m:128 * (m + 1)], reluT[:, k, :],
                             start=(k == 0), stop=(k == HC - 1))
        nc.scalar.activation(f1[:, m, :], pm_[:], AF.Relu, bias=b1_t[:, m:m + 1])
    # f2 = f1 @ w2 + b2, token-major [TC, C]
    w2_t = p_w2.tile([128, M1C, C], BF16, tag="w2p")
    nc.gpsimd.dma_start(w2_t[:], d["w2"].rearrange("(k p) f -> p k f", p=128))
    b2bc = p_w2.tile([128, C], F32, tag="b2bc")
    nc.sync.dma_start(b2bc[:], d["b2"][None, :].partition_broadcast(128)[:, 0, :])
    f2 = p_f2.tile([128, TC, CPAD], F32R, tag="f2sb")
    nc.gpsimd.memset(f2[:].bitcast(F32), 0.0)
    for c in range(TC):
        pm_ = ps_mm.tile([128, C], F32, tag="ps_mm")
        for k in range(M1C):
            nc.tensor.matmul(pm_[:], f1[:, k, 128 * c:128 * (c + 1)], w2_t[:, k, :],
                             start=(k == 0), stop=(k == M1C - 1))
        nc.vector.tensor_tensor(out=f2[:, c, 0:C], in0=pm_[:], in1=b2bc[:],
                                op=ALU.add)

    # pooling + final softmax (N padded to 428 for fp32r)
    CP2 = 428
    for s in range(BPC):
        ppool = ps_mm.tile([128, CP2], F32, tag="ps_mm")
        for j in range(2):
            pm_t = p_sm.tile([128, 128], F32R, tag="pm")
            nc.sync.dma_start(pm_t[:], pmat[256 * s + 128 * j:256 * s + 128 * (j + 1), :].bitcast(F32R))
            nc.tensor.matmul(ppool[:], pm_t[:], f2[:, 2 * s + j, 0:CP2],
                             start=(j == 0), stop=(j == 1))
        for half, src in ((0, ppool[:, 0:C]), (1, f2[:, 2 * s + 1, 0:C].bitcast(F32))):
            ex = p_scr.tile([128, CPAD], F32, tag="scr")
            se = p_sm.tile([128, 2], F32, tag="se")
            nc.scalar.activation(ex[:, 0:C], src, AF.Exp, accum_out=se[:, 0:1])
            nc.vector.reciprocal(se[:, 1:2], se[:, 0:1])
            nc.vector.tensor_scalar_mul(ex[:, 0:C], ex[:, 0:C], se[:, 1:2])
            row0 = 256 * s + 128 * half
            nc.sync.dma_start(out_d[row0:row0 + 128, :], ex[:, 0:C])


# ======================= host side =======================

_PROG_CACHE = {}


def _get_program(n_layers=L):
    if n_layers not in _PROG_CACHE:
        _PROG_CACHE[n_layers] = build_program(n_layers)
    return _PROG_CACHE[n_layers]


def make_in_maps(inputs, n_layers=L):
    """Build per-core input maps from the full-problem inputs dict."""
    f32 = lambda x: np.ascontiguousarray(np.asarray(x), dtype=np.float32)
    enc = np.asarray(inputs["encoded_batch"], dtype=np.int32)
    mask = np.asarray(inputs["mask"], dtype=np.int32)
    wpt = np.asarray(inputs["word_piece_tracked"], dtype=np.int32)

    # pooling matrix P[b, s, w] = 1/cnt[b,w] if seg[b,s]==w else 0
    cum = np.cumsum(wpt, axis=1)                      # [B, W]
    P = np.zeros((B, S, W), dtype=np.float32)
    for b in range(B):
        seg = np.searchsorted(cum[b], np.arange(S), side="right")  # [S]
        valid = seg < W
        P[b, np.arange(S)[valid], seg[valid]] = 1.0 / wpt[b, seg[valid]]

    ab = (1.0 - mask.astype(np.float32)) * -10000.0   # [B, S]

    rep = {}
    for k in ["word_emb", "pos_emb", "type_emb", "emb_ln_s", "emb_ln_b",
              "Wq", "bq", "Wk", "bk", "Wv", "bv", "Wo", "bo", "ln1_s", "ln1_b",
              "Wi", "bi", "Wo2", "bo2", "ln2_s", "ln2_b", "w1", "b1", "w2", "b2"]:
        rep[k] = f32(inputs[k])
    rep["cones"] = np.ones((128, 1), dtype=np.float32)
    rep["ceps"] = np.full((128, 1), EPS, dtype=np.float32)
    rep["crow"] = np.ones((1, 128), dtype=np.float32)

    in_maps = []
    for core in range(N_CORES):
        b0 = core * BPC
        m = dict(rep)
        m["enc"] = enc[b0:b0 + BPC].reshape(T, 1)
        m["ab"] = ab[b0:b0 + BPC].reshape(T)
        m["pmat"] = P[b0:b0 + BPC].reshape(T, W)
        in_maps.append(m)
    return in_maps


def kernel(**inputs):
    nc = _get_program(L)
    in_maps = make_in_maps(inputs, L)
    res = run_bass_kernel_spmd(nc, in_maps, core_ids=list(range(N_CORES)))
    out = np.concatenate([res.results[i]["out"].reshape(BPC, S, C)
                          for i in range(N_CORES)], axis=0)
    return out.astype(np.float32)
